# revision 7
# baseline (speedup 1.0000x reference)
# Trainium2 Bass kernel for nn_BayesianExpectationTransformerLayer.
#
# Math: attention with no positional encoding / masking is permutation-
# equivariant: _attention(x[:, perm, :]) == _attention(x)[:, perm, :].
# Hence each permuted pass, after applying the inverse permutation, equals
# the standard attention output exactly, and the whole module collapses to
#     out = c * (attention(x) @ Wo^T + bo),
#     c   = (1 - w) + w * variance_reduction_weight,
#     w   = clip(length_adaptive_weight * log(S)/S, 0.01, 1.0)
# We verify on the host that `perms` really are permutations of [0, S);
# if they are not (general fallback), we run the same device kernel once
# per pass (standard + K permuted copies) and combine on the host.
#
# Device strategy (8 NeuronCores, SPMD, tensor-parallel over heads):
#   - core c owns heads 2c, 2c+1 (feature slice F = 128 of D = 1024)
#   - per core: QT/KT = [F, B*S] projections, V in natural layout,
#     S^T = K Q^T scores per (batch, head) with the two heads packed into
#     PE row-groups, exp (softmax without max-subtraction: |scores| < 7),
#     AV with an appended ones-column producing the softmax denominator,
#     transpose+normalize fused into a matmul against diag(1/denom),
#     partial out-projection re-sharded with an AllToAll so each core
#     emits final output rows [256, 1024].
#   - host: folds scale/c into the weights, builds x^T, concatenates the
#     per-core row slices.

import os
import sys

for _p in ("/opt/trn_rl_repo", "/root/.axon_site/_ro/trn_rl_repo"):
    if os.path.isdir(_p) and _p not in sys.path:
        sys.path.append(_p)

import numpy as np

import concourse.bass as bass
import concourse.mybir as mybir
import concourse.tile as tile
from concourse import bacc
from concourse.bass import ts
from concourse.bass_utils import run_bass_kernel_spmd
from concourse.masks import make_identity

B, S, D = 2, 1024, 1024
H, HD = 16, 64
KPERM = 20
NCORES = 8
HPC = H // NCORES          # heads per core = 2
F = HPC * HD               # per-core feature slice = 128
R = B * S                  # 2048 rows
RPC = R // NCORES          # output rows per core = 256
FP32 = mybir.dt.float32

TRACE = False              # set True from test.py to capture HW profile
LAST = None                # BassKernelResults of the last run

_CACHED = None


def _build():
    """Build the SPMD Bass program (identical on all 8 cores)."""
    nc = bacc.Bacc(None)

    xT = nc.declare_dram_parameter("xT", [D, R], FP32, isOutput=False)
    wqT = nc.declare_dram_parameter("wqT", [D, F], FP32, isOutput=False)
    wkT = nc.declare_dram_parameter("wkT", [D, F], FP32, isOutput=False)
    wvT = nc.declare_dram_parameter("wvT", [D, F], FP32, isOutput=False)
    woT = nc.declare_dram_parameter("woT", [D, D], FP32, isOutput=False)
    bqs = nc.declare_dram_parameter("bqs", [F, 1], FP32, isOutput=False)
    bks = nc.declare_dram_parameter("bks", [F, 1], FP32, isOutput=False)
    bvb = nc.declare_dram_parameter("bvb", [128, HPC, HD], FP32, isOutput=False)
    out = nc.declare_dram_parameter("out", [RPC, D], FP32, isOutput=True)

    Exp = mybir.ActivationFunctionType.Exp
    NKC = S // 128           # 8 k-chunks per sequence
    NQC2 = S // 512          # 2 q-chunks of 512 per sequence
    NRC = R // 512           # 4 streamed x^T row chunks
    NRCG = R // 128          # 16 global row chunks

    with tile.TileContext(nc) as tc:
        with (
            tc.tile_pool(name="const", bufs=1) as cpool,
            tc.tile_pool(name="xt", bufs=2) as xtpool,
            tc.tile_pool(name="pt", bufs=1) as ptpool,
            tc.tile_pool(name="wo", bufs=3) as wopool,
            tc.tile_pool(name="sm", bufs=4) as smpool,
            tc.tile_pool(name="osb", bufs=2) as opool,
            tc.tile_pool(name="ps_big", bufs=4, space="PSUM") as psb,
            tc.tile_pool(name="ps_small", bufs=4, space="PSUM") as pss,
            tc.tile_pool(name="dram", bufs=1, space="DRAM") as dpool,
        ):
            # ---- constants ----
            ident = cpool.tile([128, 128], FP32, tag="ident")
            make_identity(nc, ident[:])

            wq_sb = cpool.tile([128, 8, F], FP32, tag="wq")
            wk_sb = cpool.tile([128, 8, F], FP32, tag="wk")
            wv_sb = cpool.tile([128, 8, F], FP32, tag="wv")
            nc.sync.dma_start(wq_sb[:], wqT[:].rearrange("(c p) f -> p c f", p=128))
            nc.sync.dma_start(wk_sb[:], wkT[:].rearrange("(c p) f -> p c f", p=128))
            nc.sync.dma_start(wv_sb[:], wvT[:].rearrange("(c p) f -> p c f", p=128))
            bq_sb = cpool.tile([F, 1], FP32, tag="bq")
            bk_sb = cpool.tile([F, 1], FP32, tag="bk")
            bv_sb = cpool.tile([128, HPC, HD], FP32, tag="bv")
            nc.sync.dma_start(bq_sb[:], bqs[:])
            nc.sync.dma_start(bk_sb[:], bks[:])
            nc.sync.dma_start(bv_sb[:], bvb[:])

            QT = cpool.tile([128, R], FP32, tag="QT")
            KT = cpool.tile([128, R], FP32, tag="KT")
            # V in natural layout + ones column at index HD (softmax denom).
            V0 = cpool.tile([128, NRCG, HD + 1], FP32, tag="V0")
            V1 = cpool.tile([128, NRCG, HD + 1], FP32, tag="V1")
            nc.vector.memset(V0[:, :, HD : HD + 1], 1.0)
            nc.vector.memset(V1[:, :, HD : HD + 1], 1.0)
            AT = cpool.tile([128, R], FP32, tag="AT")

            # ---- phase 1: projections ----
            xTr = xT[:].rearrange("(c p) r -> p c r", p=128)
            for rc in range(NRC):
                xt = xtpool.tile([128, 8, 512], FP32, tag="xt")
                nc.sync.dma_start(xt[:], xTr[:, :, ts(rc, 512)])
                for w_sb, b_sb, dst in ((wq_sb, bq_sb, QT), (wk_sb, bk_sb, KT)):
                    ps = psb.tile([128, 512], FP32, tag="mm512")
                    for dc in range(8):
                        nc.tensor.matmul(
                            ps[:], lhsT=w_sb[:, dc, :], rhs=xt[:, dc, :],
                            start=(dc == 0), stop=(dc == 7),
                        )
                    # dst[f, r] = ps + bias(f)
                    nc.vector.tensor_scalar_add(
                        dst[:, ts(rc, 512)], ps[:], b_sb[:, 0:1]
                    )
                for rsub in range(4):
                    ps = pss.tile([128, 128], FP32, tag="mm128")
                    for dc in range(8):
                        nc.tensor.matmul(
                            ps[:], lhsT=xt[:, dc, ts(rsub, 128)], rhs=wv_sb[:, dc, :],
                            start=(dc == 0), stop=(dc == 7),
                        )
                    rcg = rc * 4 + rsub
                    for h, Vh in ((0, V0), (1, V1)):
                        nc.vector.tensor_add(
                            Vh[:, rcg, 0:HD], ps[:, ts(h, HD)], bv_sb[:, h, :]
                        )

            # ---- phases 2+3 per batch: scores^T, exp, AV, transpose ----
            for b in range(B):
                ptb = ptpool.tile([128, HPC, NKC, S], FP32, tag="pt")
                for qc2 in range(NQC2):
                    for kc in range(NKC):
                        for h in range(HPC):
                            st = psb.tile([128, 512], FP32, tag="mm512")
                            nc.tensor.matmul(
                                st[:],
                                lhsT=KT[ts(h, HD), b * S + kc * 128 : b * S + (kc + 1) * 128],
                                rhs=QT[ts(h, HD), b * S + qc2 * 512 : b * S + (qc2 + 1) * 512],
                                start=True, stop=True,
                            )
                            nc.scalar.activation(
                                ptb[:, h, kc, ts(qc2, 512)], st[:], Exp
                            )
                for qc in range(NKC):  # 8 q chunks of 128
                    at_ps = pss.tile([128, 128], FP32, tag="mm128")
                    for h, Vh in ((0, V0), (1, V1)):
                        av = pss.tile([128, HD + 1], FP32, tag="mm128")
                        for kc in range(NKC):
                            nc.tensor.matmul(
                                av[:],
                                lhsT=ptb[:, h, kc, ts(qc, 128)],
                                rhs=Vh[:, b * NKC + kc, :],
                                start=(kc == 0), stop=(kc == 7),
                            )
                        recip = smpool.tile([128, 1], FP32, tag="recip")
                        nc.vector.reciprocal(recip[:], av[:, HD : HD + 1])
                        diag = smpool.tile([128, 128], FP32, tag="diag")
                        nc.vector.tensor_scalar_mul(diag[:], ident[:], recip[:, 0:1])
                        asb = smpool.tile([128, HD], FP32, tag="asb")
                        nc.vector.tensor_copy(asb[:], av[:, 0:HD])
                        # at_ps[64h:64h+64, q'] = A[q', f] / denom[q']
                        nc.tensor.matmul(
                            at_ps[ts(h, HD), :], lhsT=asb[:], rhs=diag[:],
                            start=True, stop=True, tile_position=(0, h * HD),
                        )
                    nc.vector.tensor_copy(
                        AT[:, b * S + qc * 128 : b * S + (qc + 1) * 128], at_ps[:]
                    )

            # ---- phase 4: AllToAll re-shard (head-split -> row-split) ----
            a2a_in = dpool.tile([NCORES, 128, RPC], FP32, tag="a2a_in")
            a2a_out = dpool.tile([NCORES, 128, RPC], FP32, tag="a2a_out")
            for j in range(NCORES):
                nc.sync.dma_start(a2a_in[j], AT[:, ts(j, RPC)])
            nc.gpsimd.collective_compute(
                "AllToAll",
                mybir.AluOpType.bypass,
                replica_groups=[list(range(NCORES))],
                ins=[a2a_in.opt()],
                outs=[a2a_out.opt()],
            )

            # ---- phase 5: out-projection on own row slice ----
            atf = cpool.tile([128, NCORES, RPC], FP32, tag="atf")
            for fc in range(NCORES):
                nc.sync.dma_start(atf[:, fc, :], a2a_out[fc])
            for dc in range(2):
                pouts = []
                for rsub in range(2):
                    po = psb.tile([128, 512], FP32, tag="mm512", name=f"po_{dc}_{rsub}")
                    pouts.append(po)
                for fc in range(8):
                    wo_t = wopool.tile([128, 512], FP32, tag="wo")
                    nc.sync.dma_start(
                        wo_t[:], woT[ts(fc, 128), ts(dc, 512)]
                    )
                    for rsub in range(2):
                        nc.tensor.matmul(
                            pouts[rsub][:],
                            lhsT=atf[:, fc, ts(rsub, 128)],
                            rhs=wo_t[:],
                            start=(fc == 0), stop=(fc == 7),
                        )
                for rsub in range(2):
                    o_sb = opool.tile([128, 512], FP32, tag="osb")
                    nc.vector.tensor_copy(o_sb[:], pouts[rsub][:])
                    nc.sync.dma_start(out[ts(rsub, 128), ts(dc, 512)], o_sb[:])

    nc.finalize()
    return nc


def _get_nc():
    global _CACHED
    if _CACHED is None:
        _CACHED = _build()
    return _CACHED


def _make_in_maps(x2d, Wq, bq, Wk, bk, Wv, bv, woT_eff):
    sm_scale = np.float32(1.0 / np.sqrt(HD))
    xT_full = np.ascontiguousarray(x2d.T).astype(np.float32, copy=False)
    woT_eff = np.ascontiguousarray(woT_eff).astype(np.float32, copy=False)

    in_maps = []
    for c in range(NCORES):
        hs = slice(c * F, (c + 1) * F)
        in_maps.append({
            "xT": xT_full,
            "wqT": np.ascontiguousarray((sm_scale * Wq[hs, :]).T),
            "wkT": np.ascontiguousarray(Wk[hs, :].T),
            "wvT": np.ascontiguousarray(Wv[hs, :].T),
            "woT": woT_eff,
            "bqs": np.ascontiguousarray((sm_scale * bq[hs])[:, None]),
            "bks": np.ascontiguousarray(bk[hs][:, None]),
            "bvb": np.ascontiguousarray(
                np.broadcast_to(bv[hs].reshape(HPC, HD)[None], (128, HPC, HD))
            ),
        })
    return in_maps


def _run_pass(x2d, Wq, bq, Wk, bk, Wv, bv, woT_eff):
    """One attention+out-projection pass on the device.

    x2d: [R, D] float32; woT_eff: [D, D] = (scale_out * Wo)^T.
    Returns [R, D] = softmax((x Wq^T + bq) (x Wk^T + bk)^T / sqrt(HD))
                     @ (x Wv^T + bv) @ (scale_out * Wo)^T  (no output bias).
    """
    global LAST
    nc = _get_nc()
    in_maps = _make_in_maps(x2d, Wq, bq, Wk, bk, Wv, bv, woT_eff)
    res = run_bass_kernel_spmd(nc, in_maps, list(range(NCORES)), trace=TRACE)
    LAST = res
    return np.concatenate([res.results[c]["out"] for c in range(NCORES)], axis=0)


def kernel(x, Wq, bq, Wk, bk, Wv, bv, Wo, bo,
           variance_reduction_weight, length_adaptive_weight, perms):
    x = np.asarray(x, dtype=np.float32)
    Wq, bq = np.asarray(Wq, np.float32), np.asarray(bq, np.float32)
    Wk, bk = np.asarray(Wk, np.float32), np.asarray(bk, np.float32)
    Wv, bv = np.asarray(Wv, np.float32), np.asarray(bv, np.float32)
    Wo, bo = np.asarray(Wo, np.float32), np.asarray(bo, np.float32)
    perms = np.asarray(perms)
    b, s, d = x.shape

    law = float(np.asarray(length_adaptive_weight).reshape(-1)[0])
    vrw = float(np.asarray(variance_reduction_weight).reshape(-1)[0])
    w = np.float32(min(max(law * np.log(s) / s, 0.01), 1.0))
    x2d = x.reshape(R, D)

    is_perm = all(
        np.array_equal(np.sort(np.asarray(perms[i])), np.arange(s))
        for i in range(perms.shape[0])
    )

    if is_perm:
        # permutation-equivariant collapse: one pass, scaled by c
        c = (1.0 - w) + w * vrw
        outp = _run_pass(x2d, Wq, bq, Wk, bk, Wv, bv, (c * Wo).T)
        outp = outp + (c * bo)[None, :]
        return outp.reshape(b, s, d).astype(np.float32)

    # general fallback: standard pass + KPERM permuted passes
    acc = _run_pass(x2d, Wq, bq, Wk, bk, Wv, bv, ((1.0 - w) * Wo).T)
    pscale = (w * vrw) / np.float32(perms.shape[0])
    for i in range(perms.shape[0]):
        perm = np.asarray(perms[i]).astype(np.int64)
        xp = x[:, perm, :].reshape(R, D)
        op = _run_pass(xp, Wq, bq, Wk, bk, Wv, bv, (pscale * Wo).T)
        op3 = op.reshape(b, s, d)
        inv = np.argsort(perm)
        acc += op3[:, inv, :].reshape(R, D)
    acc = acc + (((1.0 - w) + w * vrw) * bo)[None, :]
    return acc.reshape(b, s, d).astype(np.float32)


# revision 15
# speedup vs baseline: 8.8411x; 8.8411x over previous
# Trainium2 Bass kernel for nn_BayesianExpectationTransformerLayer.
#
# Math: attention with no positional encoding / masking is permutation-
# equivariant: _attention(x[:, perm, :]) == _attention(x)[:, perm, :].
# Hence each permuted pass, after applying the inverse permutation, equals
# the standard attention output exactly, and the whole module collapses to
#     out = c * (attention(x) @ Wo^T + bo),
#     c   = (1 - w) + w * variance_reduction_weight,
#     w   = clip(length_adaptive_weight * log(S)/S, 0.01, 1.0)
# We verify on the host that `perms` really are permutations of [0, S);
# if they are not (general fallback), we run the same device kernel once
# per pass (standard + K permuted copies) and combine on the host.
#
# Device strategy (8 NeuronCores, SPMD, tensor-parallel over heads):
#   - core c owns heads 2c, 2c+1 (feature slice F = 128 of D = 1024)
#   - per core: QT/KT = [F, B*S] projections (f32r matmuls), V via a
#     transposed projection + PE transpose, S^T = K Q^T scores per
#     (batch, head) with the two heads packed into PE row-groups,
#     exp to bf16 (softmax without max-subtraction: |scores| < 7),
#     AV in bf16 with an appended ones-column producing the softmax
#     denominator, transpose+normalize fused into a matmul against
#     diag(1/denom), two half AllToAlls re-shard head-split -> row-split
#     (overlapping compute), out-projection emits final [256, 1024] rows.
#   - host: folds scale/c into the weights, builds x^T, concatenates the
#     per-core row slices.

import os
import sys

for _p in ("/opt/trn_rl_repo", "/root/.axon_site/_ro/trn_rl_repo"):
    if os.path.isdir(_p) and _p not in sys.path:
        sys.path.append(_p)

import numpy as np

import concourse.bass as bass
import concourse.mybir as mybir
import concourse.tile as tile
from concourse import bacc
from concourse.bass import ts
from concourse.bass_utils import run_bass_kernel_spmd
from concourse.masks import make_identity

B, S, D = 2, 1024, 1024
H, HD = 16, 64
KPERM = 20
NCORES = 8
HPC = H // NCORES          # heads per core = 2
F = HPC * HD               # per-core feature slice = 128
R = B * S                  # 2048 rows
RPC = R // NCORES          # output rows per core = 256
FP32 = mybir.dt.float32
BF16 = mybir.dt.bfloat16
F32R = mybir.dt.float32r

TRACE = False              # set True from test.py to capture HW profile
LAST = None                # BassKernelResults of the last run
USE_F32R = True            # float32r for the N=512 GEMMs (full-rate fp32)

_CACHED = None


def _build():
    """Build the SPMD Bass program (identical on all 8 cores)."""
    nc = bacc.Bacc(None)

    xT = nc.declare_dram_parameter("xT", [D, R], FP32, isOutput=False)
    wqT = nc.declare_dram_parameter("wqT", [D, F], FP32, isOutput=False)
    wkT = nc.declare_dram_parameter("wkT", [D, F], FP32, isOutput=False)
    wvT = nc.declare_dram_parameter("wvT", [D, F], FP32, isOutput=False)
    woT = nc.declare_dram_parameter("woT", [D, D], FP32, isOutput=False)
    bqs = nc.declare_dram_parameter("bqs", [F, 1], FP32, isOutput=False)
    bks = nc.declare_dram_parameter("bks", [F, 1], FP32, isOutput=False)
    bvb = nc.declare_dram_parameter("bvb", [128, HPC, HD], FP32, isOutput=False)
    out = nc.declare_dram_parameter("out", [RPC, D], FP32, isOutput=True)

    Exp = mybir.ActivationFunctionType.Exp
    NKC = S // 128           # 8 k-chunks per sequence
    NQC2 = S // 512          # 2 q-chunks of 512 per sequence
    NRC = R // 512           # 4 streamed x^T row chunks
    QC_ORDER = [0, 2, 4, 6, 1, 3, 5, 7]   # evens first: feeds half-A2A early

    def fr(ap):
        return ap.bitcast(mybir.dt.float32r) if USE_F32R else ap

    with tile.TileContext(nc) as tc:
        with (
            tc.tile_pool(name="const", bufs=1) as cpool,
            tc.tile_pool(name="xt", bufs=2) as xtpool,
            tc.tile_pool(name="pt", bufs=1) as ptpool,
            tc.tile_pool(name="sm", bufs=6) as smpool,
            tc.tile_pool(name="osb", bufs=2) as opool,
            tc.tile_pool(name="ps_big", bufs=3, space="PSUM") as psb,
            tc.tile_pool(name="ps_small", bufs=5, space="PSUM") as pss,
            tc.tile_pool(name="dram", bufs=1, space="DRAM") as dpool,
        ):
            # ---- constants ----
            ident = cpool.tile([128, 128], FP32, tag="ident")
            make_identity(nc, ident[:])

            wq_sb = cpool.tile([128, 8, F], F32R, tag="wq")
            wk_sb = cpool.tile([128, 8, F], F32R, tag="wk")
            wv_sb = cpool.tile([128, 8, F], F32R, tag="wv")
            nc.gpsimd.dma_start(wq_sb[:], wqT[:].rearrange("(c p) f -> p c f", p=128))
            nc.gpsimd.dma_start(wk_sb[:], wkT[:].rearrange("(c p) f -> p c f", p=128))
            nc.gpsimd.dma_start(wv_sb[:], wvT[:].rearrange("(c p) f -> p c f", p=128))
            bq_sb = cpool.tile([F, 1], FP32, tag="bq")
            bk_sb = cpool.tile([F, 1], FP32, tag="bk")
            bv_sb = cpool.tile([128, HPC, HD], FP32, tag="bv")
            nc.sync.dma_start(bq_sb[:], bqs[:])
            nc.sync.dma_start(bk_sb[:], bks[:])
            nc.sync.dma_start(bv_sb[:], bvb[:])
            # full (c*Wo)^T staged in SBUF for the out-projection
            wof = cpool.tile([128, 8, D], F32R, tag="wof")
            nc.gpsimd.dma_start(wof[:], woT[:].rearrange("(c p) d -> p c d", p=128))

            QT = cpool.tile([128, R], F32R, tag="QT")
            KT = cpool.tile([128, R], F32R, tag="KT")
            VT = cpool.tile([128, R], FP32, tag="VT")
            # V in natural layout (bf16) + ones column at HD (softmax denom).
            V0 = cpool.tile([128, R // 128, HD + 1], BF16, tag="V0")
            V1 = cpool.tile([128, R // 128, HD + 1], BF16, tag="V1")
            nc.vector.memset(V0[:, :, HD : HD + 1], 1.0)
            nc.vector.memset(V1[:, :, HD : HD + 1], 1.0)

            # ---- phase 1: projections ----
            xTr = xT[:].rearrange("(c p) r -> p c r", p=128)
            for rc in range(NRC):
                xt = xtpool.tile([128, 8, 512], F32R, tag="xt")
                dma = nc.gpsimd
                dma.dma_start(xt[:], xTr[:, :, ts(rc, 512)])
                for w_sb, b_sb, dst in (
                    (wq_sb, bq_sb, QT), (wk_sb, bk_sb, KT), (wv_sb, None, VT)
                ):
                    ps = psb.tile([128, 512], FP32, tag="mm512", name=f"ps_{rc}")
                    for dc in range(8):
                        nc.tensor.matmul(
                            ps[:], lhsT=w_sb[:, dc, :], rhs=xt[:, dc, :],
                            start=(dc == 0), stop=(dc == 7),
                        )
                    if b_sb is not None:
                        nc.vector.tensor_scalar_add(
                            dst[:, ts(rc, 512)], ps[:], b_sb[:, 0:1]
                        )
                    else:
                        nc.vector.tensor_copy(dst[:, ts(rc, 512)], ps[:])
                # V natural: PE-transpose VT 128-blocks, add bias, cast bf16
                for rsub in range(4):
                    rcg = rc * 4 + rsub
                    tp = pss.tile([128, 128], FP32, tag="mm128", name=f"tp_{rcg}")
                    nc.tensor.transpose(
                        tp[:], VT[:, ts(rcg, 128)], ident[:]
                    )
                    for h, Vh in ((0, V0), (1, V1)):
                        nc.vector.tensor_add(
                            Vh[:, rcg, 0:HD], tp[:, ts(h, HD)], bv_sb[:, h, :]
                        )

            # ---- phases 2+3 per batch: scores^T, exp, AV, transpose ----
            # A2A staging: half 0 = first 128 rows of each 256-row block
            a2a_in0 = dpool.tile([NCORES, 128, 128], FP32, tag="a2a_in0")
            a2a_in1 = dpool.tile([NCORES, 128, 128], FP32, tag="a2a_in1")
            a2a_out0 = dpool.tile([NCORES, 128, 128], FP32, tag="a2a_out0")
            a2a_out1 = dpool.tile([NCORES, 128, 128], FP32, tag="a2a_out1")
            a2a_halves = (a2a_in0, a2a_in1)

            for b in range(B):
                ptb = ptpool.tile([128, HPC, NKC, S], BF16, tag="pt")
                for qc2 in range(NQC2):
                    for kc in range(NKC):
                        for h in range(HPC):
                            st = psb.tile(
                                [128, 512], FP32, tag="mm512",
                                name=f"st_{b}_{qc2}_{kc}_{h}",
                            )
                            nc.tensor.matmul(
                                st[:],
                                lhsT=KT[ts(h, HD), b * S + kc * 128 : b * S + (kc + 1) * 128],
                                rhs=QT[ts(h, HD), b * S + qc2 * 512 : b * S + (qc2 + 1) * 512],
                                start=True, stop=True,
                            )
                            nc.scalar.activation(
                                ptb[:, h, kc, ts(qc2, 512)], st[:], Exp
                            )
                for qc in QC_ORDER:  # 8 q chunks of 128, evens first
                    at_ps = pss.tile([128, 128], FP32, tag="mm128", name=f"at_{b}_{qc}")
                    for h, Vh in ((0, V0), (1, V1)):
                        av = pss.tile(
                            [128, HD + 1], FP32, tag="mm128", name=f"av_{b}_{qc}_{h}"
                        )
                        for kc in range(NKC):
                            nc.tensor.matmul(
                                av[:],
                                lhsT=ptb[:, h, kc, ts(qc, 128)],
                                rhs=Vh[:, b * NKC + kc, :],
                                start=(kc == 0), stop=(kc == 7),
                            )
                        recip = smpool.tile([128, 1], FP32, tag="recip")
                        nc.vector.reciprocal(recip[:], av[:, HD : HD + 1])
                        diag = smpool.tile([128, 128], FP32, tag="diag")
                        nc.vector.tensor_scalar_mul(diag[:], ident[:], recip[:, 0:1])
                        asb = smpool.tile([128, HD], FP32, tag="asb")
                        nc.vector.tensor_copy(asb[:], av[:, 0:HD])
                        # at_ps[64h:64h+64, q'] = A[q', f] / denom[q']
                        nc.tensor.matmul(
                            at_ps[ts(h, HD), :], lhsT=asb[:], rhs=diag[:],
                            start=True, stop=True, tile_position=(0, h * HD),
                        )
                    # block j of half (qc%2) <- this [128, 128] slice of A^T
                    at_sb = smpool.tile([128, 128], FP32, tag="at_sb")
                    nc.vector.tensor_copy(at_sb[:], at_ps[:])
                    j = b * 4 + qc // 2
                    nc.sync.dma_start(a2a_halves[qc % 2][j], at_sb[:])

            # ---- phase 4: two half AllToAlls (head-split -> row-split) ----
            nc.gpsimd.collective_compute(
                "AllToAll", mybir.AluOpType.bypass,
                replica_groups=[list(range(NCORES))],
                ins=[a2a_in0.opt()], outs=[a2a_out0.opt()],
            )
            nc.gpsimd.collective_compute(
                "AllToAll", mybir.AluOpType.bypass,
                replica_groups=[list(range(NCORES))],
                ins=[a2a_in1.opt()], outs=[a2a_out1.opt()],
            )

            # ---- phase 5: out-projection on own row slice ----
            for rsub, a2a_o in ((0, a2a_out0), (1, a2a_out1)):
                atf = cpool.tile([128, 8, 128], F32R, tag=f"atf{rsub}", name=f"atf{rsub}")
                nc.gpsimd.dma_start(atf[:], a2a_o[:].rearrange("c p r -> p c r"))
                for dc in range(2):
                    po = psb.tile([128, 512], FP32, tag="mm512", name=f"po_{rsub}_{dc}")
                    for fc in range(8):
                        nc.tensor.matmul(
                            po[:],
                            lhsT=atf[:, fc, :],
                            rhs=wof[:, fc, ts(dc, 512)],
                            start=(fc == 0), stop=(fc == 7),
                        )
                    o_sb = opool.tile([128, 512], FP32, tag="osb")
                    nc.vector.tensor_copy(o_sb[:], po[:])
                    nc.sync.dma_start(out[ts(rsub, 128), ts(dc, 512)], o_sb[:])

    nc.finalize()
    return nc


def _get_nc():
    global _CACHED
    if _CACHED is None:
        _CACHED = _build()
    return _CACHED


def _make_in_maps(x2d, Wq, bq, Wk, bk, Wv, bv, woT_eff):
    sm_scale = np.float32(1.0 / np.sqrt(HD))
    xT_full = np.ascontiguousarray(x2d.T).astype(np.float32, copy=False)
    woT_eff = np.ascontiguousarray(woT_eff).astype(np.float32, copy=False)

    in_maps = []
    for c in range(NCORES):
        hs = slice(c * F, (c + 1) * F)
        in_maps.append({
            "xT": xT_full,
            "wqT": np.ascontiguousarray((sm_scale * Wq[hs, :]).T),
            "wkT": np.ascontiguousarray(Wk[hs, :].T),
            "wvT": np.ascontiguousarray(Wv[hs, :].T),
            "woT": woT_eff,
            "bqs": np.ascontiguousarray((sm_scale * bq[hs])[:, None]),
            "bks": np.ascontiguousarray(bk[hs][:, None]),
            "bvb": np.ascontiguousarray(
                np.broadcast_to(bv[hs].reshape(HPC, HD)[None], (128, HPC, HD))
            ),
        })
    return in_maps


def _run_pass(x2d, Wq, bq, Wk, bk, Wv, bv, woT_eff):
    """One attention+out-projection pass on the device.

    x2d: [R, D] float32; woT_eff: [D, D] = (scale_out * Wo)^T.
    Returns [R, D] = softmax((x Wq^T + bq) (x Wk^T + bk)^T / sqrt(HD))
                     @ (x Wv^T + bv) @ (scale_out * Wo)^T  (no output bias).
    """
    global LAST
    nc = _get_nc()
    in_maps = _make_in_maps(x2d, Wq, bq, Wk, bk, Wv, bv, woT_eff)
    res = run_bass_kernel_spmd(nc, in_maps, list(range(NCORES)), trace=TRACE)
    LAST = res
    return np.concatenate([res.results[c]["out"] for c in range(NCORES)], axis=0)


def kernel(x, Wq, bq, Wk, bk, Wv, bv, Wo, bo,
           variance_reduction_weight, length_adaptive_weight, perms):
    x = np.asarray(x, dtype=np.float32)
    Wq, bq = np.asarray(Wq, np.float32), np.asarray(bq, np.float32)
    Wk, bk = np.asarray(Wk, np.float32), np.asarray(bk, np.float32)
    Wv, bv = np.asarray(Wv, np.float32), np.asarray(bv, np.float32)
    Wo, bo = np.asarray(Wo, np.float32), np.asarray(bo, np.float32)
    perms = np.asarray(perms)
    b, s, d = x.shape

    law = float(np.asarray(length_adaptive_weight).reshape(-1)[0])
    vrw = float(np.asarray(variance_reduction_weight).reshape(-1)[0])
    w = np.float32(min(max(law * np.log(s) / s, 0.01), 1.0))
    x2d = x.reshape(R, D)

    is_perm = all(
        np.array_equal(np.sort(np.asarray(perms[i])), np.arange(s))
        for i in range(perms.shape[0])
    )

    if is_perm:
        # permutation-equivariant collapse: one pass, scaled by c
        c = (1.0 - w) + w * vrw
        outp = _run_pass(x2d, Wq, bq, Wk, bk, Wv, bv, (c * Wo).T)
        outp = outp + (c * bo)[None, :]
        return outp.reshape(b, s, d).astype(np.float32)

    # general fallback: standard pass + KPERM permuted passes
    acc = _run_pass(x2d, Wq, bq, Wk, bk, Wv, bv, ((1.0 - w) * Wo).T)
    pscale = (w * vrw) / np.float32(perms.shape[0])
    for i in range(perms.shape[0]):
        perm = np.asarray(perms[i]).astype(np.int64)
        xp = x[:, perm, :].reshape(R, D)
        op = _run_pass(xp, Wq, bq, Wk, bk, Wv, bv, (pscale * Wo).T)
        op3 = op.reshape(b, s, d)
        inv = np.argsort(perm)
        acc += op3[:, inv, :].reshape(R, D)
    acc = acc + (((1.0 - w) + w * vrw) * bo)[None, :]
    return acc.reshape(b, s, d).astype(np.float32)
